# revision 18
# baseline (speedup 1.0000x reference)
"""GCN 2-layer forward for 8 TRN2 NeuronCores — matmul-aggregation design.

Per core (node row-partition by dst, degree-balanced):
  Layer 1: host pre-gathers X[src] per edge (pure layout) -> kernel streams
    message chunks sequentially (no SWDGE), builds weighted one-hot M per
    128-token chunk in one fused DVE op, and segment-sums via PSUM-accumulated
    matmuls:  Z^T[F,128dst] += msg[128tok,F](stationary)^T @ M[128tok,128dst].
  Dense: H = relu(Z @ W) straight from Z^T (no transpose needed); BN stats
    accumulated per tile, reduced via ones-matmul + tiny AllReduce.
  Layer 2: h1 (bf16, post-BN) AllGathered into a shared table; messages
    gathered per edge with SWDGE dma_gather (2 queues), same M-matmul
    aggregation; BN2; f32 output.

Pad tokens carry dstloc=-1 -> M row exactly zero -> zero contribution.
"""
import sys

sys.path.insert(0, "/opt/trn_rl_repo")
import numpy as np
import ml_dtypes
import concourse.bass as bass
import concourse.bacc as bacc
import concourse.mybir as mybir
from concourse import tile
from concourse.masks import make_identity

F32 = mybir.dt.float32
BF16 = mybir.dt.bfloat16
I16 = mybir.dt.int16
AF = mybir.ActivationFunctionType
ALU = mybir.AluOpType


class Cfg2:
    def __init__(self, N, E, H2, H1, D, P=8, half=32768, cap=1024, nq=2,
                 eps=1e-5):
        self.N, self.E, self.H2, self.H1, self.D, self.P = N, E, H2, H1, D, P
        self.half, self.cap, self.nq, self.eps = half, cap, nq, eps
        bins = -(-N // 128)                   # 128-node bins, balanced degree
        self.NT = -(-bins // P)               # tiles per core
        self.NTP = self.NT * 128              # padded rows per core
        self.NTP_EXT = self.NTP + 16          # +16 rows carry BN1 partial sums
        self.NPHYS = P * self.NTP_EXT         # global padded table rows
        self.SPLIT_T = max(1, (self.NT * 2) // 3)  # AllGather slice boundary (tiles)
        self.S1 = self.SPLIT_T * 128          # slice-1 rows per core
        self.S2 = self.NTP_EXT - self.S1      # slice-2 rows per core (+stats)
        assert self.NPHYS - half < 32768, "hi half exceeds int16 range"


def build_plan2(cfg, edge_src, edge_dst, edge_weight, y_features):
    P, NT, NTP, half = cfg.P, cfg.NT, cfg.NTP_EXT, cfg.half
    N, E = cfg.N, cfg.E
    edge_src = np.asarray(edge_src).astype(np.int64)
    edge_dst = np.asarray(edge_dst).astype(np.int64)
    w_all = np.asarray(edge_weight).astype(np.float32)
    X = np.asarray(y_features).astype(np.float32)

    # degree-balanced node -> (core, tile, slot) assignment
    deg = np.bincount(edge_dst, minlength=N)
    order = np.argsort(-deg, kind="stable")
    nbins = P * NT
    phys = np.empty(N, np.int64)
    bin_fill = np.zeros(nbins, np.int64)
    S1, S2 = cfg.S1, cfg.S2
    SPLIT_T = cfg.SPLIT_T
    for i, node in enumerate(order):
        rnd, pos = divmod(i, nbins)
        b = pos if rnd % 2 == 0 else nbins - 1 - pos   # serpentine
        core, t = divmod(b, NT)
        r = t * 128 + bin_fill[b]
        if r < S1:
            phys[node] = core * S1 + r
        else:
            phys[node] = P * S1 + core * S2 + (r - S1)
        bin_fill[b] += 1
    assert bin_fill.max() <= 128

    srcp = phys[edge_src]
    dstp = phys[edge_dst]

    # per-core, per-tile, per-half edge lists
    in_s1 = dstp < P * S1
    core_of = np.where(in_s1, dstp // S1, (dstp - P * S1) // S2)
    row_of = np.where(in_s1, dstp % S1, S1 + (dstp - P * S1) % S2)
    tile_of = row_of // 128
    slot_of = row_of % 128
    hi_of = (srcp >= half).astype(np.int64)

    cnt = np.zeros((P, NT, 2), np.int64)
    buckets = {}
    for c in range(P):
        m = core_of == c
        s, t, sl, h, w = srcp[m], tile_of[m], slot_of[m], hi_of[m], w_all[m]
        for tt in range(NT):
            tm = t == tt
            for hh in (0, 1):
                hm = tm & (h == hh)
                buckets[(c, tt, hh)] = (s[hm], sl[hm], w[hm])
                cnt[c, tt, hh] = hm.sum()

    Csec = np.zeros((NT, 2), np.int64)     # chunks per (tile, half), shared
    for tt in range(NT):
        for hh in (0, 1):
            Csec[tt, hh] = -(-cnt[:, tt, hh].max() // 128)
    C_total = int(Csec.sum())
    TOKT = C_total * 128

    # global chunk schedule: per tile, lo chunks then hi chunks
    # calls: per (tile, half) section, split into <=cap-token calls
    calls = []          # (half, tok_off, ntok, sec_tile, sec_start_chunk)
    chunk_tile = []     # per global chunk: tile index
    chunk_call = []     # per global chunk: call index
    off = 0
    for tt in range(NT):
        for hh in (0, 1):
            nch = int(Csec[tt, hh])
            done = 0
            while done < nch:
                take = min(cfg.cap // 128, nch - done)
                calls.append((hh, off + done * 128, take * 128, tt, done))
                for k in range(take):
                    chunk_tile.append(tt)
                    chunk_call.append(len(calls) - 1)
                done += take
            off += nch * 128

    # per-core packed arrays
    xp = np.zeros((cfg.NPHYS, cfg.H2), np.float32)
    xp[phys] = X

    def wrap16(a):
        return np.tile(a.reshape(-1, 16).T, (8, 1)).copy()

    per_core = []
    for c in range(P):
        gidx = np.zeros(TOKT, np.int16)
        dstloc = np.full(TOKT, -1.0, np.float32)
        wts = np.zeros(TOKT, np.float32)
        msrc = np.zeros(TOKT, np.int64)    # phys row per token (pad -> 0)
        o = 0
        for tt in range(NT):
            for hh in (0, 1):
                s, sl, w = buckets[(c, tt, hh)]
                n = len(s)
                gidx[o:o + n] = (s - (half if hh else 0)).astype(np.int16)
                dstloc[o:o + n] = sl
                wts[o:o + n] = w
                msrc[o:o + n] = s
                o += int(Csec[tt, hh]) * 128
        F1 = cfg.H2 + 1
        m1 = np.ones((TOKT, F1), np.float32)
        m1[:, :cfg.H2] = xp[msrc]
        m1[wts == 0, :cfg.H2] = 0
        m1 = m1.astype(ml_dtypes.bfloat16)
        per_core.append({
            "gidx": wrap16(gidx),
            "dstloc": dstloc.reshape(-1, 128).T.astype(ml_dtypes.bfloat16).copy(),
            "wts": wts.reshape(-1, 128).T.astype(ml_dtypes.bfloat16).copy(),
            # m1 layout [128 part, C_total*F1]: token (ci,p) row at [p, ci*F1:]
            "m1": np.ascontiguousarray(
                m1.reshape(C_total, 128, F1).transpose(1, 0, 2)
                .reshape(128, C_total * F1)),
        })

    plan = {
        "Csec": Csec, "C_total": C_total, "TOKT": TOKT, "calls": calls,
        "chunk_tile": chunk_tile, "chunk_call": chunk_call, "phys": phys,
    }
    return plan, per_core


def build_bass2(cfg, plan):
    P, NT, NTP, NPHYS = cfg.P, cfg.NT, cfg.NTP, cfg.NPHYS
    NTPE = cfg.NTP_EXT
    H2, H1, D, half, eps, N = cfg.H2, cfg.H1, cfg.D, cfg.half, cfg.eps, cfg.N
    Csec, C_total, TOKT = plan["Csec"], plan["C_total"], plan["TOKT"]
    calls = plan["calls"]
    CmaxT = int((Csec[:, 0] + Csec[:, 1]).max())     # L1 stream tile chunks
    CmaxC = max(ntok // 128 for (_, _, ntok, _, _) in calls)  # L2 call chunks

    nc = bacc.Bacc("TRN2", target_bir_lowering=False, debug=False,
                   num_swdge_queues=cfg.nq)

    m1 = nc.dram_tensor("m1", [128, C_total * (H2 + 1)], BF16, kind="ExternalInput")
    gidx = nc.dram_tensor("gidx", [128, TOKT // 16], I16, kind="ExternalInput")
    dstloc = nc.dram_tensor("dstloc", [128, C_total], BF16, kind="ExternalInput")
    wts = nc.dram_tensor("wts", [128, C_total], BF16, kind="ExternalInput")
    iota_in = nc.dram_tensor("iota", [128, 128], BF16, kind="ExternalInput")
    mask16 = nc.dram_tensor("mask16", [16, 2], BF16, kind="ExternalInput")
    w1 = nc.dram_tensor("w1", [H2, H1], F32, kind="ExternalInput")
    w2 = nc.dram_tensor("w2", [H1, D], F32, kind="ExternalInput")
    gam1 = nc.dram_tensor("gam1", [H1], F32, kind="ExternalInput")
    bet1 = nc.dram_tensor("bet1", [H1], F32, kind="ExternalInput")
    gam2 = nc.dram_tensor("gam2", [D], F32, kind="ExternalInput")
    bet2 = nc.dram_tensor("bet2", [D], F32, kind="ExternalInput")
    out = nc.dram_tensor("out", [NTP, D], F32, kind="ExternalOutput")

    h1my = nc.dram_tensor("h1my", [NTPE, H1], BF16)
    h1full = nc.dram_tensor("h1full", [NPHYS, H1], BF16, addr_space="Shared")
    bn2_in = nc.dram_tensor("bn2_in", [1, 2 * D], F32)
    bn2_out = nc.dram_tensor("bn2_out", [1, 2 * D], F32, addr_space="Shared")

    groups = [list(range(P))]

    with tile.TileContext(nc) as tc:
        with (
            tc.tile_pool(name="const", bufs=1) as constp,
            tc.tile_pool(name="stream", bufs=3) as sp,
            tc.tile_pool(name="gath", bufs=3) as gp,
            tc.tile_pool(name="mm", bufs=3) as mp,
            tc.tile_pool(name="dense", bufs=2) as dp,
            tc.tile_pool(name="bn", bufs=1) as bnp,
            tc.tile_pool(name="psz", bufs=3, space="PSUM") as ppz,
            tc.tile_pool(name="psh", bufs=2, space="PSUM") as pph,
            tc.tile_pool(name="psb", bufs=1, space="PSUM") as ppb,
        ):
            # ---- resident constants ----
            iota_sb = constp.tile([128, 128], BF16)
            nc.sync.dma_start(out=iota_sb[:], in_=iota_in.ap())
            dst_sb = constp.tile([128, C_total], BF16)
            nc.sync.dma_start(out=dst_sb[:], in_=dstloc.ap())
            w_sb = constp.tile([128, C_total], BF16)
            nc.sync.dma_start(out=w_sb[:], in_=wts.ap())
            gi_sb = constp.tile([128, TOKT // 16], I16)
            w1_sb = constp.tile([H2, H1], F32)
            nc.sync.dma_start(out=w1_sb[:], in_=w1.ap())
            w2_sb = constp.tile([H1, D], F32)
            ones = constp.tile([128, 1], F32)
            nc.vector.memset(ones[:], 1.0)
            eps_t = constp.tile([128, 1], F32)
            nc.vector.memset(eps_t[:], eps)
            ident = constp.tile([128, 128], F32)
            make_identity(nc, ident[:])
            gb1 = constp.tile([2, H1], F32)
            nc.sync.dma_start(out=gb1[0:1, :], in_=gam1.ap().unsqueeze(0))
            nc.sync.dma_start(out=gb1[1:2, :], in_=bet1.ap().unsqueeze(0))
            mask16_sb = constp.tile([16, 2], BF16)
            nc.sync.dma_start(out=mask16_sb[:], in_=mask16.ap())
            gam2_b = constp.tile([128, D], F32)
            bet2_b = constp.tile([128, D], F32)
            nc.sync.dma_start(out=gam2_b[:], in_=gam2.ap().unsqueeze(0).broadcast_to([128, D]))
            nc.sync.dma_start(out=bet2_b[:], in_=bet2.ap().unsqueeze(0).broadcast_to([128, D]))
            acc1 = constp.tile([128, 2 * H1], F32)
            acc2 = constp.tile([128, 2 * D], F32)
            nc.vector.memset(acc1[:], 0.0)
            nc.vector.memset(acc2[:], 0.0)
            hraw2 = constp.tile([128, NT * D], F32)
            degw_all = constp.tile([1, NT * 128], F32)

            # tile -> (chunk offsets): rebuild per-tile schedule
            tile_chunks = [[] for _ in range(NT)]
            for gci, tt in enumerate(plan["chunk_tile"]):
                tile_chunks[tt].append(gci)
            call_of_chunk = plan["chunk_call"]

            def layer(lnum, Fin, Fout, w_t, acc, hraw, table, s2v_sb=None):
                qsel = [0]
                loaded_call = [-1]
                cur = {}

                def build_M(c0, C, tag):
                    Mb = mp.tile([128, CmaxT if tag == "Mb1" else CmaxC, 128],
                                 BF16, tag=tag)
                    dbc = dst_sb[:, c0:c0 + C].unsqueeze(-1).broadcast_to([128, C, 128])
                    wbc = w_sb[:, c0:c0 + C].unsqueeze(-1).broadcast_to([128, C, 128])
                    ibc = iota_sb[:].unsqueeze(1).broadcast_to([128, C, 128])
                    nc.vector.tensor_tensor(Mb[:, :C, :], dbc, ibc, ALU.is_equal)
                    nc.vector.tensor_tensor(Mb[:, :C, :], Mb[:, :C, :], wbc, ALU.mult)
                    return Mb

                Fst = Fin + 1 if lnum == 1 else Fin

                def prefetch1(tt):
                    chs = tile_chunks[tt]
                    c0 = chs[0]
                    nch = len(chs)
                    st = sp.tile([128, CmaxT, Fst], BF16, tag="m1t")
                    nc.sync.dma_start(
                        out=st[:, :nch, :],
                        in_=m1.ap()[:, c0 * Fst:(c0 + nch) * Fst]
                            .rearrange("p (c f) -> p c f", f=Fst))
                    return st, build_M(c0, nch, "Mb1")

                pend = []
                pf = []
                if lnum == 1:
                    pf.append(prefetch1(0))
                    if NT > 1:
                        pf.append(prefetch1(1))
                for tt in range(NT):
                    chs = tile_chunks[tt]
                    if not chs:
                        continue
                    zt_full = ppz.tile([128, 128], F32, tag="zt")
                    zt_ps = zt_full[:Fst, :]
                    if lnum == 1:
                        st, Mb1 = pf.pop(0)
                        if tt + 2 < NT:
                            pf.append(prefetch1(tt + 2))
                    for ki, gci in enumerate(chs):
                        if lnum == 2:
                            ci = call_of_chunk[gci]
                            if ci != loaded_call[0]:
                                hh, tok_off, ntok, _, _ = calls[ci]
                                gt = gp.tile([128, CmaxC, Fin], BF16, tag="g2")
                                src_ap = table.ap()[half:, :] if hh else table.ap()
                                nc.gpsimd.dma_gather(
                                    out_ap=gt[:, :ntok // 128, :], in_ap=src_ap,
                                    idxs_ap=gi_sb[:, tok_off // 16:(tok_off + ntok) // 16],
                                    num_idxs=ntok, num_idxs_reg=ntok,
                                    elem_size=Fin, queue_num=qsel[0],
                                    single_packet=False)
                                qsel[0] = (qsel[0] + 1) % cfg.nq
                                loaded_call[0] = ci
                                cur["tile"] = gt
                                cur["base"] = gci
                                cur["M"] = build_M(gci, ntok // 128, "Mb2")
                            msg = cur["tile"][:, gci - cur["base"], :]
                            M = cur["M"][:, gci - cur["base"], :]
                        else:
                            msg = st[:, ki, :]
                            M = Mb1[:, ki, :]
                        nc.tensor.matmul(zt_ps, msg, M,
                                         start=(ki == 0), stop=(ki == len(chs) - 1))
                    # dense: H = relu(Z @ W) (+ degw x s2v correction in L2)
                    if lnum == 1:
                        pend.append((tt, zt_full))
                        if len(pend) < 3:
                            continue
                        tt, zt_full = pend.pop(0)
                        zt_ps = zt_full[:Fst, :]
                    ztsb_full = dp.tile([128, 128], F32, tag="ztsb")
                    nc.scalar.copy(ztsb_full[:Fst, :], zt_ps)
                    if lnum == 1:
                        nc.sync.dma_start(
                            out=degw_all[:, tt * 128:(tt + 1) * 128],
                            in_=ztsb_full[Fin:Fin + 1, :])
                    h_full = pph.tile([128, 256], F32, tag="h")
                    h_ps = h_full[:, :Fout]
                    nc.tensor.matmul(h_ps, ztsb_full[:Fin, :], w_t[:],
                                     start=True, stop=(lnum == 1))
                    if lnum == 2:
                        nc.tensor.matmul(h_ps, degw_all[:, tt * 128:(tt + 1) * 128],
                                         s2v_sb[:], start=False, stop=True)
                    hview = hraw[:, tt * Fout:(tt + 1) * Fout] if lnum == 2 else None
                    if lnum == 1:
                        hb = dp.tile([128, H1], BF16, tag="hbf")
                        nc.scalar.activation(hb[:], h_ps, AF.Relu)
                        sq = dp.tile([128, Fout], F32, tag="sq")
                        nc.scalar.activation(sq[:], hb[:], AF.Square)
                        nc.gpsimd.tensor_add(acc[:, :Fout], acc[:, :Fout], hb[:])
                        nc.gpsimd.tensor_add(acc[:, Fout:], acc[:, Fout:], sq[:])
                        nc.sync.dma_start(
                            out=h1my.ap()[tt * 128:(tt + 1) * 128, :], in_=hb[:])
                        if tt == cfg.SPLIT_T - 1:
                            nc.gpsimd.collective_compute(
                                "AllGather", ALU.bypass, replica_groups=groups,
                                ins=[h1my.ap()[:cfg.S1, :]],
                                outs=[h1full.ap()[:P * cfg.S1, :]])
                    else:
                        nc.scalar.activation(hview, h_ps, AF.Relu)
                        sq = dp.tile([128, Fout], F32, tag="sq")
                        nc.scalar.activation(sq[:], hview, AF.Square)
                        nc.vector.tensor_add(acc[:, :Fout], acc[:, :Fout], hview)
                        nc.vector.tensor_add(acc[:, Fout:], acc[:, Fout:], sq[:])
                while lnum == 1 and pend:
                    tt, zt_full = pend.pop(0)
                    zt_ps = zt_full[:Fst, :]
                    ztsb_full = dp.tile([128, 128], F32, tag="ztsb")
                    nc.scalar.copy(ztsb_full[:Fst, :], zt_ps)
                    nc.sync.dma_start(
                        out=degw_all[:, tt * 128:(tt + 1) * 128],
                        in_=ztsb_full[Fin:Fin + 1, :])
                    h_full = pph.tile([128, 256], F32, tag="h")
                    h_ps = h_full[:, :Fout]
                    nc.tensor.matmul(h_ps, ztsb_full[:Fin, :], w_t[:],
                                     start=True, stop=True)
                    hb = dp.tile([128, H1], BF16, tag="hbf")
                    nc.scalar.activation(hb[:], h_ps, AF.Relu)
                    sq = dp.tile([128, Fout], F32, tag="sq")
                    nc.scalar.activation(sq[:], hb[:], AF.Square)
                    nc.gpsimd.tensor_add(acc[:, :Fout], acc[:, :Fout], hb[:])
                    nc.gpsimd.tensor_add(acc[:, Fout:], acc[:, Fout:], sq[:])
                    nc.sync.dma_start(
                        out=h1my.ap()[tt * 128:(tt + 1) * 128, :], in_=hb[:])
                    if tt == cfg.SPLIT_T - 1:
                        nc.gpsimd.collective_compute(
                            "AllGather", ALU.bypass, replica_groups=groups,
                            ins=[h1my.ap()[:cfg.S1, :]],
                            outs=[h1full.ap()[:P * cfg.S1, :]])

            def bn_stats(F, acc, bn_in, bn_out, tag):
                st_ps = ppb.tile([1, 512], F32, tag="st")
                nc.tensor.matmul(st_ps[:, :2 * F], ones[:], acc[:], start=True, stop=True)
                st_sb = dp.tile([1, 2 * F], F32, tag=f"st{tag}")
                nc.scalar.copy(st_sb[:], st_ps[:, :2 * F])
                nc.sync.dma_start(out=bn_in.ap(), in_=st_sb[:])
                nc.gpsimd.collective_compute(
                    "AllReduce", ALU.add, replica_groups=groups,
                    ins=[bn_in.ap()], outs=[bn_out.ap()])
                stb = bnp.tile([128, 2 * F], F32, tag=f"stb{tag}")
                nc.sync.dma_start(out=stb[:], in_=bn_out.ap().broadcast_to([128, 2 * F]))
                mean = bnp.tile([128, F], F32, tag=f"mean{tag}")
                var = bnp.tile([128, F], F32, tag=f"var{tag}")
                nc.scalar.mul(mean[:], stb[:, :F], 1.0 / N)
                nc.scalar.mul(var[:], stb[:, F:], 1.0 / N)
                msq = bnp.tile([128, F], F32, tag=f"msq{tag}")
                nc.scalar.activation(msq[:], mean[:], AF.Square)
                nc.vector.tensor_sub(var[:], var[:], msq[:])
                sd = bnp.tile([128, F], F32, tag=f"sd{tag}")
                nc.scalar.activation(sd[:], var[:], AF.Sqrt, bias=eps_t[:])
                inv = bnp.tile([128, F], F32, tag=f"inv{tag}")
                nc.vector.reciprocal(inv[:], sd[:])
                return mean, inv

            # ================= layer 1 =================
            layer(1, H2, H1, w1_sb, acc1, None, None)
            nc.sync.dma_start(out=gi_sb[:], in_=gidx.ap())
            nc.sync.dma_start(out=w2_sb[:], in_=w2.ap())
            # append this core's BN1 partial sums as 2 bf16 rows of h1my
            st_ps = ppb.tile([1, 512], F32, tag="st")
            nc.tensor.matmul(st_ps[:, :2 * H1], ones[:], acc1[:], start=True, stop=True)
            st_bf = dp.tile([1, 2 * H1], BF16, tag="stbf")
            nc.scalar.copy(st_bf[:], st_ps[:, :2 * H1])
            nc.sync.dma_start(out=h1my.ap()[NTP:NTP + 1, :], in_=st_bf[:, :H1])
            nc.sync.dma_start(out=h1my.ap()[NTP + 1:NTP + 2, :], in_=st_bf[:, H1:])
            nc.gpsimd.collective_compute(
                "AllGather", ALU.bypass, replica_groups=groups,
                ins=[h1my.ap()[cfg.S1:, :]],
                outs=[h1full.ap()[P * cfg.S1:, :]])
            # combine the 8 partial-stat row pairs locally (no AllReduce)
            stat_off = P * cfg.S1 + (NTP - cfg.S1)
            statall = bnp.tile([16, H1], BF16, tag="statall")
            for cc in range(P):
                nc.sync.dma_start(
                    out=statall[2 * cc:2 * cc + 2, :],
                    in_=h1full.ap()[stat_off + cc * cfg.S2:
                                    stat_off + cc * cfg.S2 + 2, :])
            stred_ps = ppb.tile([2, H1], F32, tag="str")
            nc.tensor.matmul(stred_ps[:], mask16_sb[:], statall[:], start=True, stop=True)
            stsb = dp.tile([2, H1], F32, tag="stsb")
            nc.scalar.copy(stsb[:], stred_ps[:])
            tr_ps = ppb.tile([128, 4], F32, tag="tr")
            nc.tensor.transpose(tr_ps[:H1, 0:2], stsb[:], ident[:2, 0:2])
            colstats = bnp.tile([H1, 2], F32, tag="colstats")
            nc.scalar.copy(colstats[:], tr_ps[:H1, 0:2])
            nc.tensor.transpose(tr_ps[:H1, 2:4], gb1[:], ident[:2, 0:2])
            gbc = bnp.tile([H1, 2], F32, tag="gbc")
            nc.scalar.copy(gbc[:], tr_ps[:H1, 2:4])
            gam1_c = gbc[:, 0:1]
            bet1_c = gbc[:, 1:2]
            mean_c = bnp.tile([H1, 1], F32, tag="mean_c")
            var_c = bnp.tile([H1, 1], F32, tag="var_c")
            nc.scalar.mul(mean_c[:], colstats[:, 0:1], 1.0 / N)
            nc.scalar.mul(var_c[:], colstats[:, 1:2], 1.0 / N)
            msq_c = bnp.tile([H1, 1], F32, tag="msq_c")
            nc.scalar.activation(msq_c[:], mean_c[:], AF.Square)
            nc.vector.tensor_sub(var_c[:], var_c[:], msq_c[:])
            sd_c = bnp.tile([H1, 1], F32, tag="sd_c")
            nc.scalar.activation(sd_c[:], var_c[:], AF.Sqrt, bias=eps_t[:H1])
            inv_c = bnp.tile([H1, 1], F32, tag="inv_c")
            nc.vector.reciprocal(inv_c[:], sd_c[:])
            scale_c = bnp.tile([H1, 1], F32, tag="scale_c")
            shift_c = bnp.tile([H1, 1], F32, tag="shift_c")
            nc.vector.tensor_mul(scale_c[:], gam1_c, inv_c[:])
            nc.vector.tensor_mul(shift_c[:], mean_c[:], scale_c[:])
            nc.vector.tensor_sub(shift_c[:], bet1_c, shift_c[:])
            w2p = constp.tile([H1, D], F32)
            nc.vector.tensor_scalar_mul(w2p[:], w2_sb[:], scale_c[:])
            s2v_ps = ppb.tile([1, 512], F32, tag="st")
            nc.tensor.matmul(s2v_ps[:, :D], shift_c[:], w2_sb[:], start=True, stop=True)
            s2v_sb = bnp.tile([1, D], F32, tag="s2v")
            nc.scalar.copy(s2v_sb[:], s2v_ps[:, :D])
            # ================= layer 2 =================
            layer(2, H1, D, w2p, acc2, hraw2, h1full, s2v_sb=s2v_sb)
            mean2, inv2 = bn_stats(D, acc2, bn2_in, bn2_out, "2")
            scale2 = bnp.tile([128, D], F32, tag="scale2")
            shift2 = bnp.tile([128, D], F32, tag="shift2")
            nc.vector.tensor_mul(scale2[:], gam2_b[:], inv2[:])
            nc.vector.tensor_mul(shift2[:], mean2[:], scale2[:])
            nc.vector.tensor_sub(shift2[:], bet2_b[:], shift2[:])
            GAPP = 13
            for g0 in range(0, NT, GAPP):
                gn = min(GAPP, NT - g0)
                hv = hraw2[:, g0 * D:(g0 + gn) * D].rearrange(
                    "p (t f) -> p t f", f=D)
                t2 = dp.tile([128, GAPP, D], F32, tag="bna2")
                sc_b = scale2[:].unsqueeze(1).broadcast_to([128, gn, D])
                sh_b = shift2[:].unsqueeze(1).broadcast_to([128, gn, D])
                nc.vector.tensor_mul(t2[:, :gn, :], hv, sc_b)
                nc.vector.tensor_add(t2[:, :gn, :], t2[:, :gn, :], sh_b)
                nc.sync.dma_start(
                    out=out.ap()[g0 * 128:(g0 + gn) * 128, :]
                        .rearrange("(t p) f -> p t f", p=128),
                    in_=t2[:, :gn, :])

    nc.compile()
    return nc


def make_in_maps2(cfg, plan, per_core, inputs):
    iota = np.tile(np.arange(128, dtype=np.float32), (128, 1)).astype(ml_dtypes.bfloat16)
    mask16 = np.zeros((16, 2), np.float32)
    mask16[np.arange(16), np.arange(16) % 2] = 1.0
    shared = {
        "iota": iota,
        "mask16": mask16.astype(ml_dtypes.bfloat16),
        "w1": np.ascontiguousarray(inputs["W1"], dtype=np.float32),
        "w2": np.ascontiguousarray(inputs["W2"], dtype=np.float32),
        "gam1": np.ascontiguousarray(inputs["gamma1"], dtype=np.float32),
        "bet1": np.ascontiguousarray(inputs["beta1"], dtype=np.float32),
        "gam2": np.ascontiguousarray(inputs["gamma2"], dtype=np.float32),
        "bet2": np.ascontiguousarray(inputs["beta2"], dtype=np.float32),
    }
    return [{**shared, **per_core[c]} for c in range(cfg.P)]


def run2(cfg, inputs, use_hw=True, trace=False):
    plan, per_core = build_plan2(
        cfg, inputs["edge_src"], inputs["edge_dst"], inputs["edge_weight"],
        inputs["y_features"])
    print(f"plan2: C_total={plan['C_total']} chunks ({plan['TOKT']} tokens, "
          f"real {cfg.E // cfg.P} avg), {len(plan['calls'])} calls/layer")
    nc = build_bass2(cfg, plan)
    in_maps = make_in_maps2(cfg, plan, per_core, inputs)
    phys = plan["phys"]
    if use_hw:
        from concourse.bass_utils import run_bass_kernel_spmd
        res = run_bass_kernel_spmd(nc, in_maps, list(range(cfg.P)), trace=trace)
        outs = np.concatenate([res.results[c]["out"] for c in range(cfg.P)], 0)
    else:
        from concourse.bass_interp import MultiCoreSim
        sim = MultiCoreSim(nc, num_cores=cfg.P, require_finite=False,
                           require_nnan=False)
        for c in range(cfg.P):
            for k, v in in_maps[c].items():
                sim.cores[c].tensor(k)[:] = v
            sim.cores[c].tensor("out")[:] = 0
        sim.simulate()
        outs = np.concatenate(
            [np.array(sim.cores[c].mem_tensor("out")) for c in range(cfg.P)], 0)
        res = None
    in_s1 = phys < cfg.P * cfg.S1
    core = np.where(in_s1, phys // cfg.S1, (phys - cfg.P * cfg.S1) // cfg.S2)
    row = np.where(in_s1, phys % cfg.S1,
                   cfg.S1 + (phys - cfg.P * cfg.S1) % cfg.S2)
    idx = core * cfg.NTP + row
    return np.ascontiguousarray(outs[idx], dtype=np.float32), res


# ======================= harness entry point =======================

_CFG = Cfg2(N=50000, E=800000, H2=64, H1=128, D=256, P=8,
            half=32768, cap=1024, nq=2)


def kernel(**inputs) -> np.ndarray:
    """Full-input, full-output 2-layer GCN forward on 8 TRN2 NeuronCores."""
    out, _ = run2(_CFG, inputs, use_hw=True, trace=False)
    return np.ascontiguousarray(out, dtype=np.float32)


# revision 19
# speedup vs baseline: 1.1548x; 1.1548x over previous
"""GCN 2-layer forward for 8 TRN2 NeuronCores — matmul-aggregation design.

Per core (node row-partition by dst, degree-balanced):
  Layer 1: host pre-gathers X[src] per edge (pure layout) -> kernel streams
    message chunks sequentially (no SWDGE), builds weighted one-hot M per
    128-token chunk in one fused DVE op, and segment-sums via PSUM-accumulated
    matmuls:  Z^T[F,128dst] += msg[128tok,F](stationary)^T @ M[128tok,128dst].
  Dense: H = relu(Z @ W) straight from Z^T (no transpose needed); BN stats
    accumulated per tile, reduced via ones-matmul + tiny AllReduce.
  Layer 2: h1 (bf16, post-BN) AllGathered into a shared table; messages
    gathered per edge with SWDGE dma_gather (2 queues), same M-matmul
    aggregation; BN2; f32 output.

Pad tokens carry dstloc=-1 -> M row exactly zero -> zero contribution.
"""
import sys

sys.path.insert(0, "/opt/trn_rl_repo")
import numpy as np
import ml_dtypes
import concourse.bass as bass
import concourse.bacc as bacc
import concourse.mybir as mybir
from concourse import tile
from concourse.masks import make_identity

F32 = mybir.dt.float32
BF16 = mybir.dt.bfloat16
I16 = mybir.dt.int16
AF = mybir.ActivationFunctionType
ALU = mybir.AluOpType


class Cfg2:
    def __init__(self, N, E, H2, H1, D, P=8, half=32768, cap=1024, nq=2,
                 eps=1e-5):
        self.N, self.E, self.H2, self.H1, self.D, self.P = N, E, H2, H1, D, P
        self.half, self.cap, self.nq, self.eps = half, cap, nq, eps
        bins = -(-N // 128)                   # 128-node bins, balanced degree
        self.NT = -(-bins // P)               # tiles per core
        self.NTP = self.NT * 128              # padded rows per core
        self.NTP_EXT = self.NTP + 16          # +16 rows carry BN1 partial sums
        self.NPHYS = P * self.NTP_EXT         # global padded table rows
        self.SPLIT_T = (self.NT + 1) // 2     # AllGather slice boundary (tiles)
        self.S1 = self.SPLIT_T * 128          # slice-1 rows per core
        self.S2 = self.NTP_EXT - self.S1      # slice-2 rows per core (+stats)
        assert self.NPHYS - half < 32768, "hi half exceeds int16 range"


def build_plan2(cfg, edge_src, edge_dst, edge_weight, y_features):
    P, NT, NTP, half = cfg.P, cfg.NT, cfg.NTP_EXT, cfg.half
    N, E = cfg.N, cfg.E
    edge_src = np.asarray(edge_src).astype(np.int64)
    edge_dst = np.asarray(edge_dst).astype(np.int64)
    w_all = np.asarray(edge_weight).astype(np.float32)
    X = np.asarray(y_features).astype(np.float32)

    # degree-balanced node -> (core, tile, slot) assignment
    deg = np.bincount(edge_dst, minlength=N)
    order = np.argsort(-deg, kind="stable")
    nbins = P * NT
    phys = np.empty(N, np.int64)
    bin_fill = np.zeros(nbins, np.int64)
    S1, S2 = cfg.S1, cfg.S2
    SPLIT_T = cfg.SPLIT_T
    for i, node in enumerate(order):
        rnd, pos = divmod(i, nbins)
        b = pos if rnd % 2 == 0 else nbins - 1 - pos   # serpentine
        core, t = divmod(b, NT)
        r = t * 128 + bin_fill[b]
        if r < S1:
            phys[node] = core * S1 + r
        else:
            phys[node] = P * S1 + core * S2 + (r - S1)
        bin_fill[b] += 1
    assert bin_fill.max() <= 128

    srcp = phys[edge_src]
    dstp = phys[edge_dst]

    # per-core, per-tile, per-half edge lists
    in_s1 = dstp < P * S1
    core_of = np.where(in_s1, dstp // S1, (dstp - P * S1) // S2)
    row_of = np.where(in_s1, dstp % S1, S1 + (dstp - P * S1) % S2)
    tile_of = row_of // 128
    slot_of = row_of % 128
    hi_of = (srcp >= half).astype(np.int64)

    cnt = np.zeros((P, NT, 2), np.int64)
    buckets = {}
    for c in range(P):
        m = core_of == c
        s, t, sl, h, w = srcp[m], tile_of[m], slot_of[m], hi_of[m], w_all[m]
        for tt in range(NT):
            tm = t == tt
            for hh in (0, 1):
                hm = tm & (h == hh)
                buckets[(c, tt, hh)] = (s[hm], sl[hm], w[hm])
                cnt[c, tt, hh] = hm.sum()

    Csec = np.zeros((NT, 2), np.int64)     # chunks per (tile, half), shared
    for tt in range(NT):
        for hh in (0, 1):
            Csec[tt, hh] = -(-cnt[:, tt, hh].max() // 128)
    C_total = int(Csec.sum())
    TOKT = C_total * 128

    # global chunk schedule: per tile, lo chunks then hi chunks
    # calls: per (tile, half) section, split into <=cap-token calls
    calls = []          # (half, tok_off, ntok, sec_tile, sec_start_chunk)
    chunk_tile = []     # per global chunk: tile index
    chunk_call = []     # per global chunk: call index
    off = 0
    for tt in range(NT):
        for hh in (0, 1):
            nch = int(Csec[tt, hh])
            done = 0
            while done < nch:
                take = min(cfg.cap // 128, nch - done)
                calls.append((hh, off + done * 128, take * 128, tt, done))
                for k in range(take):
                    chunk_tile.append(tt)
                    chunk_call.append(len(calls) - 1)
                done += take
            off += nch * 128

    # per-core packed arrays
    xp = np.zeros((cfg.NPHYS, cfg.H2), np.float32)
    xp[phys] = X

    def wrap16(a):
        return np.tile(a.reshape(-1, 16).T, (8, 1)).copy()

    per_core = []
    for c in range(P):
        gidx = np.zeros(TOKT, np.int16)
        dstloc = np.full(TOKT, -1.0, np.float32)
        wts = np.zeros(TOKT, np.float32)
        msrc = np.zeros(TOKT, np.int64)    # phys row per token (pad -> 0)
        o = 0
        for tt in range(NT):
            for hh in (0, 1):
                s, sl, w = buckets[(c, tt, hh)]
                n = len(s)
                gidx[o:o + n] = (s - (half if hh else 0)).astype(np.int16)
                dstloc[o:o + n] = sl
                wts[o:o + n] = w
                msrc[o:o + n] = s
                o += int(Csec[tt, hh]) * 128
        F1 = cfg.H2 + 1
        m1 = np.ones((TOKT, F1), np.float32)
        m1[:, :cfg.H2] = xp[msrc]
        m1[wts == 0, :cfg.H2] = 0
        m1 = m1.astype(ml_dtypes.bfloat16)
        per_core.append({
            "gidx": wrap16(gidx),
            "dstloc": dstloc.reshape(-1, 128).T.astype(ml_dtypes.bfloat16).copy(),
            "wts": wts.reshape(-1, 128).T.astype(ml_dtypes.bfloat16).copy(),
            # m1 layout [128 part, C_total*F1]: token (ci,p) row at [p, ci*F1:]
            "m1": np.ascontiguousarray(
                m1.reshape(C_total, 128, F1).transpose(1, 0, 2)
                .reshape(128, C_total * F1)),
        })

    plan = {
        "Csec": Csec, "C_total": C_total, "TOKT": TOKT, "calls": calls,
        "chunk_tile": chunk_tile, "chunk_call": chunk_call, "phys": phys,
    }
    return plan, per_core


def build_bass2(cfg, plan):
    P, NT, NTP, NPHYS = cfg.P, cfg.NT, cfg.NTP, cfg.NPHYS
    NTPE = cfg.NTP_EXT
    H2, H1, D, half, eps, N = cfg.H2, cfg.H1, cfg.D, cfg.half, cfg.eps, cfg.N
    Csec, C_total, TOKT = plan["Csec"], plan["C_total"], plan["TOKT"]
    calls = plan["calls"]
    CmaxT = int((Csec[:, 0] + Csec[:, 1]).max())     # L1 stream tile chunks
    CmaxC = max(ntok // 128 for (_, _, ntok, _, _) in calls)  # L2 call chunks

    nc = bacc.Bacc("TRN2", target_bir_lowering=False, debug=False,
                   num_swdge_queues=cfg.nq)

    m1 = nc.dram_tensor("m1", [128, C_total * (H2 + 1)], BF16, kind="ExternalInput")
    gidx = nc.dram_tensor("gidx", [128, TOKT // 16], I16, kind="ExternalInput")
    dstloc = nc.dram_tensor("dstloc", [128, C_total], BF16, kind="ExternalInput")
    wts = nc.dram_tensor("wts", [128, C_total], BF16, kind="ExternalInput")
    iota_in = nc.dram_tensor("iota", [128, 128], BF16, kind="ExternalInput")
    mask16 = nc.dram_tensor("mask16", [16, 2], BF16, kind="ExternalInput")
    w1 = nc.dram_tensor("w1", [H2, H1], F32, kind="ExternalInput")
    w2 = nc.dram_tensor("w2", [H1, D], F32, kind="ExternalInput")
    gam1 = nc.dram_tensor("gam1", [H1], F32, kind="ExternalInput")
    bet1 = nc.dram_tensor("bet1", [H1], F32, kind="ExternalInput")
    gam2 = nc.dram_tensor("gam2", [D], F32, kind="ExternalInput")
    bet2 = nc.dram_tensor("bet2", [D], F32, kind="ExternalInput")
    out = nc.dram_tensor("out", [NTP, D], F32, kind="ExternalOutput")

    h1my = nc.dram_tensor("h1my", [NTPE, H1], BF16)
    h1full = nc.dram_tensor("h1full", [NPHYS, H1], BF16, addr_space="Shared")
    bn2_in = nc.dram_tensor("bn2_in", [1, 2 * D], F32)
    bn2_out = nc.dram_tensor("bn2_out", [1, 2 * D], F32, addr_space="Shared")

    groups = [list(range(P))]

    with tile.TileContext(nc) as tc:
        with (
            tc.tile_pool(name="const", bufs=1) as constp,
            tc.tile_pool(name="stream", bufs=3) as sp,
            tc.tile_pool(name="gath", bufs=3) as gp,
            tc.tile_pool(name="mm", bufs=3) as mp,
            tc.tile_pool(name="dense", bufs=2) as dp,
            tc.tile_pool(name="bn", bufs=1) as bnp,
            tc.tile_pool(name="psz", bufs=3, space="PSUM") as ppz,
            tc.tile_pool(name="psh", bufs=2, space="PSUM") as pph,
            tc.tile_pool(name="psb", bufs=1, space="PSUM") as ppb,
        ):
            # ---- resident constants ----
            iota_sb = constp.tile([128, 128], BF16)
            nc.sync.dma_start(out=iota_sb[:], in_=iota_in.ap())
            dst_sb = constp.tile([128, C_total], BF16)
            nc.sync.dma_start(out=dst_sb[:], in_=dstloc.ap())
            w_sb = constp.tile([128, C_total], BF16)
            nc.sync.dma_start(out=w_sb[:], in_=wts.ap())
            gi_sb = constp.tile([128, TOKT // 16], I16)
            w1_sb = constp.tile([H2, H1], F32)
            nc.sync.dma_start(out=w1_sb[:], in_=w1.ap())
            w2_sb = constp.tile([H1, D], F32)
            ones = constp.tile([128, 1], F32)
            nc.vector.memset(ones[:], 1.0)
            eps_t = constp.tile([128, 1], F32)
            nc.vector.memset(eps_t[:], eps)
            ident = constp.tile([128, 128], F32)
            make_identity(nc, ident[:])
            gb1 = constp.tile([2, H1], F32)
            nc.sync.dma_start(out=gb1[0:1, :], in_=gam1.ap().unsqueeze(0))
            nc.sync.dma_start(out=gb1[1:2, :], in_=bet1.ap().unsqueeze(0))
            mask16_sb = constp.tile([16, 2], BF16)
            nc.sync.dma_start(out=mask16_sb[:], in_=mask16.ap())
            gam2_b = constp.tile([128, D], F32)
            bet2_b = constp.tile([128, D], F32)
            nc.sync.dma_start(out=gam2_b[:], in_=gam2.ap().unsqueeze(0).broadcast_to([128, D]))
            nc.sync.dma_start(out=bet2_b[:], in_=bet2.ap().unsqueeze(0).broadcast_to([128, D]))
            acc1 = constp.tile([128, 2 * H1], F32)
            acc2 = constp.tile([128, 2 * D], F32)
            nc.vector.memset(acc1[:], 0.0)
            nc.vector.memset(acc2[:], 0.0)
            hraw2 = constp.tile([128, NT * D], F32)
            degw_all = constp.tile([1, NT * 128], F32)

            # tile -> (chunk offsets): rebuild per-tile schedule
            tile_chunks = [[] for _ in range(NT)]
            for gci, tt in enumerate(plan["chunk_tile"]):
                tile_chunks[tt].append(gci)
            call_of_chunk = plan["chunk_call"]

            def layer(lnum, Fin, Fout, w_t, acc, hraw, table, s2v_sb=None):
                qsel = [0]
                loaded_call = [-1]
                cur = {}

                def build_M(c0, C, tag):
                    Mb = mp.tile([128, CmaxT if tag == "Mb1" else CmaxC, 128],
                                 BF16, tag=tag)
                    dbc = dst_sb[:, c0:c0 + C].unsqueeze(-1).broadcast_to([128, C, 128])
                    wbc = w_sb[:, c0:c0 + C].unsqueeze(-1).broadcast_to([128, C, 128])
                    ibc = iota_sb[:].unsqueeze(1).broadcast_to([128, C, 128])
                    nc.vector.tensor_tensor(Mb[:, :C, :], dbc, ibc, ALU.is_equal)
                    nc.vector.tensor_tensor(Mb[:, :C, :], Mb[:, :C, :], wbc, ALU.mult)
                    return Mb

                Fst = Fin + 1 if lnum == 1 else Fin

                def prefetch1(tt):
                    chs = tile_chunks[tt]
                    c0 = chs[0]
                    nch = len(chs)
                    st = sp.tile([128, CmaxT, Fst], BF16, tag="m1t")
                    nc.sync.dma_start(
                        out=st[:, :nch, :],
                        in_=m1.ap()[:, c0 * Fst:(c0 + nch) * Fst]
                            .rearrange("p (c f) -> p c f", f=Fst))
                    return st, build_M(c0, nch, "Mb1")

                pend = []
                pf = []
                if lnum == 1:
                    pf.append(prefetch1(0))
                    if NT > 1:
                        pf.append(prefetch1(1))
                for tt in range(NT):
                    chs = tile_chunks[tt]
                    if not chs:
                        continue
                    zt_full = ppz.tile([128, 128], F32, tag="zt")
                    zt_ps = zt_full[:Fst, :]
                    if lnum == 1:
                        st, Mb1 = pf.pop(0)
                        if tt + 2 < NT:
                            pf.append(prefetch1(tt + 2))
                    for ki, gci in enumerate(chs):
                        if lnum == 2:
                            ci = call_of_chunk[gci]
                            if ci != loaded_call[0]:
                                hh, tok_off, ntok, _, _ = calls[ci]
                                gt = gp.tile([128, CmaxC, Fin], BF16, tag="g2")
                                src_ap = table.ap()[half:, :] if hh else table.ap()
                                nc.gpsimd.dma_gather(
                                    out_ap=gt[:, :ntok // 128, :], in_ap=src_ap,
                                    idxs_ap=gi_sb[:, tok_off // 16:(tok_off + ntok) // 16],
                                    num_idxs=ntok, num_idxs_reg=ntok,
                                    elem_size=Fin, queue_num=qsel[0],
                                    single_packet=False)
                                qsel[0] = (qsel[0] + 1) % cfg.nq
                                loaded_call[0] = ci
                                cur["tile"] = gt
                                cur["base"] = gci
                                cur["M"] = build_M(gci, ntok // 128, "Mb2")
                            msg = cur["tile"][:, gci - cur["base"], :]
                            M = cur["M"][:, gci - cur["base"], :]
                        else:
                            msg = st[:, ki, :]
                            M = Mb1[:, ki, :]
                        nc.tensor.matmul(zt_ps, msg, M,
                                         start=(ki == 0), stop=(ki == len(chs) - 1))
                    # dense: H = relu(Z @ W) (+ degw x s2v correction in L2)
                    if lnum == 1:
                        pend.append((tt, zt_full))
                        if len(pend) < 3:
                            continue
                        tt, zt_full = pend.pop(0)
                        zt_ps = zt_full[:Fst, :]
                    ztsb_full = dp.tile([128, 128], F32, tag="ztsb")
                    nc.scalar.copy(ztsb_full[:Fst, :], zt_ps)
                    if lnum == 1:
                        nc.sync.dma_start(
                            out=degw_all[:, tt * 128:(tt + 1) * 128],
                            in_=ztsb_full[Fin:Fin + 1, :])
                    h_full = pph.tile([128, 256], F32, tag="h")
                    h_ps = h_full[:, :Fout]
                    nc.tensor.matmul(h_ps, ztsb_full[:Fin, :], w_t[:],
                                     start=True, stop=(lnum == 1))
                    if lnum == 2:
                        nc.tensor.matmul(h_ps, degw_all[:, tt * 128:(tt + 1) * 128],
                                         s2v_sb[:], start=False, stop=True)
                    hview = hraw[:, tt * Fout:(tt + 1) * Fout] if lnum == 2 else None
                    if lnum == 1:
                        hb = dp.tile([128, H1], BF16, tag="hbf")
                        nc.scalar.activation(hb[:], h_ps, AF.Relu)
                        sq = dp.tile([128, Fout], F32, tag="sq")
                        nc.scalar.activation(sq[:], hb[:], AF.Square)
                        nc.gpsimd.tensor_add(acc[:, :Fout], acc[:, :Fout], hb[:])
                        nc.gpsimd.tensor_add(acc[:, Fout:], acc[:, Fout:], sq[:])
                        nc.sync.dma_start(
                            out=h1my.ap()[tt * 128:(tt + 1) * 128, :], in_=hb[:])
                        if tt == cfg.SPLIT_T - 1:
                            nc.gpsimd.collective_compute(
                                "AllGather", ALU.bypass, replica_groups=groups,
                                ins=[h1my.ap()[:cfg.S1, :]],
                                outs=[h1full.ap()[:P * cfg.S1, :]])
                    else:
                        nc.scalar.activation(hview, h_ps, AF.Relu)
                        sq = dp.tile([128, Fout], F32, tag="sq")
                        nc.scalar.activation(sq[:], hview, AF.Square)
                        nc.vector.tensor_add(acc[:, :Fout], acc[:, :Fout], hview)
                        nc.vector.tensor_add(acc[:, Fout:], acc[:, Fout:], sq[:])
                while lnum == 1 and pend:
                    tt, zt_full = pend.pop(0)
                    zt_ps = zt_full[:Fst, :]
                    ztsb_full = dp.tile([128, 128], F32, tag="ztsb")
                    nc.scalar.copy(ztsb_full[:Fst, :], zt_ps)
                    nc.sync.dma_start(
                        out=degw_all[:, tt * 128:(tt + 1) * 128],
                        in_=ztsb_full[Fin:Fin + 1, :])
                    h_full = pph.tile([128, 256], F32, tag="h")
                    h_ps = h_full[:, :Fout]
                    nc.tensor.matmul(h_ps, ztsb_full[:Fin, :], w_t[:],
                                     start=True, stop=True)
                    hb = dp.tile([128, H1], BF16, tag="hbf")
                    nc.scalar.activation(hb[:], h_ps, AF.Relu)
                    sq = dp.tile([128, Fout], F32, tag="sq")
                    nc.scalar.activation(sq[:], hb[:], AF.Square)
                    nc.gpsimd.tensor_add(acc[:, :Fout], acc[:, :Fout], hb[:])
                    nc.gpsimd.tensor_add(acc[:, Fout:], acc[:, Fout:], sq[:])
                    nc.sync.dma_start(
                        out=h1my.ap()[tt * 128:(tt + 1) * 128, :], in_=hb[:])
                    if tt == cfg.SPLIT_T - 1:
                        nc.gpsimd.collective_compute(
                            "AllGather", ALU.bypass, replica_groups=groups,
                            ins=[h1my.ap()[:cfg.S1, :]],
                            outs=[h1full.ap()[:P * cfg.S1, :]])

            def bn_stats(F, acc, bn_in, bn_out, tag):
                st_ps = ppb.tile([1, 512], F32, tag="st")
                nc.tensor.matmul(st_ps[:, :2 * F], ones[:], acc[:], start=True, stop=True)
                st_sb = dp.tile([1, 2 * F], F32, tag=f"st{tag}")
                nc.scalar.copy(st_sb[:], st_ps[:, :2 * F])
                nc.sync.dma_start(out=bn_in.ap(), in_=st_sb[:])
                nc.gpsimd.collective_compute(
                    "AllReduce", ALU.add, replica_groups=groups,
                    ins=[bn_in.ap()], outs=[bn_out.ap()])
                stb = bnp.tile([128, 2 * F], F32, tag=f"stb{tag}")
                nc.sync.dma_start(out=stb[:], in_=bn_out.ap().broadcast_to([128, 2 * F]))
                mean = bnp.tile([128, F], F32, tag=f"mean{tag}")
                var = bnp.tile([128, F], F32, tag=f"var{tag}")
                nc.scalar.mul(mean[:], stb[:, :F], 1.0 / N)
                nc.scalar.mul(var[:], stb[:, F:], 1.0 / N)
                msq = bnp.tile([128, F], F32, tag=f"msq{tag}")
                nc.scalar.activation(msq[:], mean[:], AF.Square)
                nc.vector.tensor_sub(var[:], var[:], msq[:])
                sd = bnp.tile([128, F], F32, tag=f"sd{tag}")
                nc.scalar.activation(sd[:], var[:], AF.Sqrt, bias=eps_t[:])
                inv = bnp.tile([128, F], F32, tag=f"inv{tag}")
                nc.vector.reciprocal(inv[:], sd[:])
                return mean, inv

            # ================= layer 1 =================
            layer(1, H2, H1, w1_sb, acc1, None, None)
            nc.sync.dma_start(out=gi_sb[:], in_=gidx.ap())
            nc.sync.dma_start(out=w2_sb[:], in_=w2.ap())
            # append this core's BN1 partial sums as 2 bf16 rows of h1my
            st_ps = ppb.tile([1, 512], F32, tag="st")
            nc.tensor.matmul(st_ps[:, :2 * H1], ones[:], acc1[:], start=True, stop=True)
            st_bf = dp.tile([1, 2 * H1], BF16, tag="stbf")
            nc.scalar.copy(st_bf[:], st_ps[:, :2 * H1])
            nc.sync.dma_start(out=h1my.ap()[NTP:NTP + 1, :], in_=st_bf[:, :H1])
            nc.sync.dma_start(out=h1my.ap()[NTP + 1:NTP + 2, :], in_=st_bf[:, H1:])
            nc.gpsimd.collective_compute(
                "AllGather", ALU.bypass, replica_groups=groups,
                ins=[h1my.ap()[cfg.S1:, :]],
                outs=[h1full.ap()[P * cfg.S1:, :]])
            # combine the 8 partial-stat row pairs locally (no AllReduce)
            stat_off = P * cfg.S1 + (NTP - cfg.S1)
            statall = bnp.tile([16, H1], BF16, tag="statall")
            for cc in range(P):
                nc.sync.dma_start(
                    out=statall[2 * cc:2 * cc + 2, :],
                    in_=h1full.ap()[stat_off + cc * cfg.S2:
                                    stat_off + cc * cfg.S2 + 2, :])
            stred_ps = ppb.tile([2, H1], F32, tag="str")
            nc.tensor.matmul(stred_ps[:], mask16_sb[:], statall[:], start=True, stop=True)
            stsb = dp.tile([2, H1], F32, tag="stsb")
            nc.scalar.copy(stsb[:], stred_ps[:])
            tr_ps = ppb.tile([128, 4], F32, tag="tr")
            nc.tensor.transpose(tr_ps[:H1, 0:2], stsb[:], ident[:2, 0:2])
            colstats = bnp.tile([H1, 2], F32, tag="colstats")
            nc.scalar.copy(colstats[:], tr_ps[:H1, 0:2])
            nc.tensor.transpose(tr_ps[:H1, 2:4], gb1[:], ident[:2, 0:2])
            gbc = bnp.tile([H1, 2], F32, tag="gbc")
            nc.scalar.copy(gbc[:], tr_ps[:H1, 2:4])
            gam1_c = gbc[:, 0:1]
            bet1_c = gbc[:, 1:2]
            mean_c = bnp.tile([H1, 1], F32, tag="mean_c")
            var_c = bnp.tile([H1, 1], F32, tag="var_c")
            nc.scalar.mul(mean_c[:], colstats[:, 0:1], 1.0 / N)
            nc.scalar.mul(var_c[:], colstats[:, 1:2], 1.0 / N)
            msq_c = bnp.tile([H1, 1], F32, tag="msq_c")
            nc.scalar.activation(msq_c[:], mean_c[:], AF.Square)
            nc.vector.tensor_sub(var_c[:], var_c[:], msq_c[:])
            sd_c = bnp.tile([H1, 1], F32, tag="sd_c")
            nc.scalar.activation(sd_c[:], var_c[:], AF.Sqrt, bias=eps_t[:H1])
            inv_c = bnp.tile([H1, 1], F32, tag="inv_c")
            nc.vector.reciprocal(inv_c[:], sd_c[:])
            scale_c = bnp.tile([H1, 1], F32, tag="scale_c")
            shift_c = bnp.tile([H1, 1], F32, tag="shift_c")
            nc.vector.tensor_mul(scale_c[:], gam1_c, inv_c[:])
            nc.vector.tensor_mul(shift_c[:], mean_c[:], scale_c[:])
            nc.vector.tensor_sub(shift_c[:], bet1_c, shift_c[:])
            w2p = constp.tile([H1, D], F32)
            nc.vector.tensor_scalar_mul(w2p[:], w2_sb[:], scale_c[:])
            s2v_ps = ppb.tile([1, 512], F32, tag="st")
            nc.tensor.matmul(s2v_ps[:, :D], shift_c[:], w2_sb[:], start=True, stop=True)
            s2v_sb = bnp.tile([1, D], F32, tag="s2v")
            nc.scalar.copy(s2v_sb[:], s2v_ps[:, :D])
            # ================= layer 2 =================
            layer(2, H1, D, w2p, acc2, hraw2, h1full, s2v_sb=s2v_sb)
            mean2, inv2 = bn_stats(D, acc2, bn2_in, bn2_out, "2")
            scale2 = bnp.tile([128, D], F32, tag="scale2")
            shift2 = bnp.tile([128, D], F32, tag="shift2")
            nc.vector.tensor_mul(scale2[:], gam2_b[:], inv2[:])
            nc.vector.tensor_mul(shift2[:], mean2[:], scale2[:])
            nc.vector.tensor_sub(shift2[:], bet2_b[:], shift2[:])
            GAPP = 13
            for g0 in range(0, NT, GAPP):
                gn = min(GAPP, NT - g0)
                hv = hraw2[:, g0 * D:(g0 + gn) * D].rearrange(
                    "p (t f) -> p t f", f=D)
                t2 = dp.tile([128, GAPP, D], F32, tag="bna2")
                sc_b = scale2[:].unsqueeze(1).broadcast_to([128, gn, D])
                sh_b = shift2[:].unsqueeze(1).broadcast_to([128, gn, D])
                nc.vector.tensor_mul(t2[:, :gn, :], hv, sc_b)
                nc.vector.tensor_add(t2[:, :gn, :], t2[:, :gn, :], sh_b)
                nc.sync.dma_start(
                    out=out.ap()[g0 * 128:(g0 + gn) * 128, :]
                        .rearrange("(t p) f -> p t f", p=128),
                    in_=t2[:, :gn, :])

    nc.compile()
    return nc


def make_in_maps2(cfg, plan, per_core, inputs):
    iota = np.tile(np.arange(128, dtype=np.float32), (128, 1)).astype(ml_dtypes.bfloat16)
    mask16 = np.zeros((16, 2), np.float32)
    mask16[np.arange(16), np.arange(16) % 2] = 1.0
    shared = {
        "iota": iota,
        "mask16": mask16.astype(ml_dtypes.bfloat16),
        "w1": np.ascontiguousarray(inputs["W1"], dtype=np.float32),
        "w2": np.ascontiguousarray(inputs["W2"], dtype=np.float32),
        "gam1": np.ascontiguousarray(inputs["gamma1"], dtype=np.float32),
        "bet1": np.ascontiguousarray(inputs["beta1"], dtype=np.float32),
        "gam2": np.ascontiguousarray(inputs["gamma2"], dtype=np.float32),
        "bet2": np.ascontiguousarray(inputs["beta2"], dtype=np.float32),
    }
    return [{**shared, **per_core[c]} for c in range(cfg.P)]


def run2(cfg, inputs, use_hw=True, trace=False):
    plan, per_core = build_plan2(
        cfg, inputs["edge_src"], inputs["edge_dst"], inputs["edge_weight"],
        inputs["y_features"])
    print(f"plan2: C_total={plan['C_total']} chunks ({plan['TOKT']} tokens, "
          f"real {cfg.E // cfg.P} avg), {len(plan['calls'])} calls/layer")
    nc = build_bass2(cfg, plan)
    in_maps = make_in_maps2(cfg, plan, per_core, inputs)
    phys = plan["phys"]
    if use_hw:
        from concourse.bass_utils import run_bass_kernel_spmd
        res = run_bass_kernel_spmd(nc, in_maps, list(range(cfg.P)), trace=trace)
        outs = np.concatenate([res.results[c]["out"] for c in range(cfg.P)], 0)
    else:
        from concourse.bass_interp import MultiCoreSim
        sim = MultiCoreSim(nc, num_cores=cfg.P, require_finite=False,
                           require_nnan=False)
        for c in range(cfg.P):
            for k, v in in_maps[c].items():
                sim.cores[c].tensor(k)[:] = v
            sim.cores[c].tensor("out")[:] = 0
        sim.simulate()
        outs = np.concatenate(
            [np.array(sim.cores[c].mem_tensor("out")) for c in range(cfg.P)], 0)
        res = None
    in_s1 = phys < cfg.P * cfg.S1
    core = np.where(in_s1, phys // cfg.S1, (phys - cfg.P * cfg.S1) // cfg.S2)
    row = np.where(in_s1, phys % cfg.S1,
                   cfg.S1 + (phys - cfg.P * cfg.S1) % cfg.S2)
    idx = core * cfg.NTP + row
    return np.ascontiguousarray(outs[idx], dtype=np.float32), res


# ======================= harness entry point =======================

_CFG = Cfg2(N=50000, E=800000, H2=64, H1=128, D=256, P=8,
            half=32768, cap=1024, nq=2)


def kernel(**inputs) -> np.ndarray:
    """Full-input, full-output 2-layer GCN forward on 8 TRN2 NeuronCores."""
    out, _ = run2(_CFG, inputs, use_hw=True, trace=False)
    return np.ascontiguousarray(out, dtype=np.float32)


# revision 20
# speedup vs baseline: 1.1701x; 1.0132x over previous
"""GCN 2-layer forward for 8 TRN2 NeuronCores — matmul-aggregation design.

Per core (node row-partition by dst, degree-balanced):
  Layer 1: host pre-gathers X[src] per edge (pure layout) -> kernel streams
    message chunks sequentially (no SWDGE), builds weighted one-hot M per
    128-token chunk in one fused DVE op, and segment-sums via PSUM-accumulated
    matmuls:  Z^T[F,128dst] += msg[128tok,F](stationary)^T @ M[128tok,128dst].
  Dense: H = relu(Z @ W) straight from Z^T (no transpose needed); BN stats
    accumulated per tile, reduced via ones-matmul + tiny AllReduce.
  Layer 2: h1 (bf16, post-BN) AllGathered into a shared table; messages
    gathered per edge with SWDGE dma_gather (2 queues), same M-matmul
    aggregation; BN2; f32 output.

Pad tokens carry dstloc=-1 -> M row exactly zero -> zero contribution.
"""
import sys

sys.path.insert(0, "/opt/trn_rl_repo")
import numpy as np
import ml_dtypes
import concourse.bass as bass
import concourse.bacc as bacc
import concourse.mybir as mybir
from concourse import tile
from concourse.masks import make_identity

F32 = mybir.dt.float32
BF16 = mybir.dt.bfloat16
I16 = mybir.dt.int16
AF = mybir.ActivationFunctionType
ALU = mybir.AluOpType


class Cfg2:
    def __init__(self, N, E, H2, H1, D, P=8, half=32768, cap=1024, nq=2,
                 eps=1e-5):
        self.N, self.E, self.H2, self.H1, self.D, self.P = N, E, H2, H1, D, P
        self.half, self.cap, self.nq, self.eps = half, cap, nq, eps
        bins = -(-N // 128)                   # 128-node bins, balanced degree
        self.NT = -(-bins // P)               # tiles per core
        self.NTP = self.NT * 128              # padded rows per core
        self.NTP_EXT = self.NTP + 16          # +16 rows carry BN1 partial sums
        self.NPHYS = P * self.NTP_EXT         # global padded table rows
        self.SPLIT_T = max(1, (self.NT * 2) // 3)  # AllGather slice boundary (tiles)
        self.S1 = self.SPLIT_T * 128          # slice-1 rows per core
        self.S2 = self.NTP_EXT - self.S1      # slice-2 rows per core (+stats)
        assert self.NPHYS - half < 32768, "hi half exceeds int16 range"


def build_plan2(cfg, edge_src, edge_dst, edge_weight, y_features):
    P, NT, NTP, half = cfg.P, cfg.NT, cfg.NTP_EXT, cfg.half
    N, E = cfg.N, cfg.E
    edge_src = np.asarray(edge_src).astype(np.int64)
    edge_dst = np.asarray(edge_dst).astype(np.int64)
    w_all = np.asarray(edge_weight).astype(np.float32)
    X = np.asarray(y_features).astype(np.float32)

    # degree-balanced node -> (core, tile, slot) assignment
    deg = np.bincount(edge_dst, minlength=N)
    order = np.argsort(-deg, kind="stable")
    nbins = P * NT
    phys = np.empty(N, np.int64)
    bin_fill = np.zeros(nbins, np.int64)
    S1, S2 = cfg.S1, cfg.S2
    SPLIT_T = cfg.SPLIT_T
    for i, node in enumerate(order):
        rnd, pos = divmod(i, nbins)
        b = pos if rnd % 2 == 0 else nbins - 1 - pos   # serpentine
        core, t = divmod(b, NT)
        r = t * 128 + bin_fill[b]
        if r < S1:
            phys[node] = core * S1 + r
        else:
            phys[node] = P * S1 + core * S2 + (r - S1)
        bin_fill[b] += 1
    assert bin_fill.max() <= 128

    srcp = phys[edge_src]
    dstp = phys[edge_dst]

    # per-core, per-tile, per-half edge lists
    in_s1 = dstp < P * S1
    core_of = np.where(in_s1, dstp // S1, (dstp - P * S1) // S2)
    row_of = np.where(in_s1, dstp % S1, S1 + (dstp - P * S1) % S2)
    tile_of = row_of // 128
    slot_of = row_of % 128
    hi_of = (srcp >= half).astype(np.int64)

    cnt = np.zeros((P, NT, 2), np.int64)
    buckets = {}
    for c in range(P):
        m = core_of == c
        s, t, sl, h, w = srcp[m], tile_of[m], slot_of[m], hi_of[m], w_all[m]
        for tt in range(NT):
            tm = t == tt
            for hh in (0, 1):
                hm = tm & (h == hh)
                buckets[(c, tt, hh)] = (s[hm], sl[hm], w[hm])
                cnt[c, tt, hh] = hm.sum()

    Csec = np.zeros((NT, 2), np.int64)     # chunks per (tile, half), shared
    for tt in range(NT):
        for hh in (0, 1):
            Csec[tt, hh] = -(-cnt[:, tt, hh].max() // 128)
    C_total = int(Csec.sum())
    TOKT = C_total * 128

    # global chunk schedule: per tile, lo chunks then hi chunks
    # calls: per (tile, half) section, split into <=cap-token calls
    calls = []          # (half, tok_off, ntok, sec_tile, sec_start_chunk)
    chunk_tile = []     # per global chunk: tile index
    chunk_call = []     # per global chunk: call index
    off = 0
    for tt in range(NT):
        for hh in (0, 1):
            nch = int(Csec[tt, hh])
            done = 0
            while done < nch:
                take = min(cfg.cap // 128, nch - done)
                calls.append((hh, off + done * 128, take * 128, tt, done))
                for k in range(take):
                    chunk_tile.append(tt)
                    chunk_call.append(len(calls) - 1)
                done += take
            off += nch * 128

    # per-core packed arrays
    xp = np.zeros((cfg.NPHYS, cfg.H2), np.float32)
    xp[phys] = X

    def wrap16(a):
        return np.tile(a.reshape(-1, 16).T, (8, 1)).copy()

    per_core = []
    for c in range(P):
        gidx = np.zeros(TOKT, np.int16)
        dstloc = np.full(TOKT, -1.0, np.float32)
        wts = np.zeros(TOKT, np.float32)
        msrc = np.zeros(TOKT, np.int64)    # phys row per token (pad -> 0)
        o = 0
        for tt in range(NT):
            for hh in (0, 1):
                s, sl, w = buckets[(c, tt, hh)]
                n = len(s)
                gidx[o:o + n] = (s - (half if hh else 0)).astype(np.int16)
                dstloc[o:o + n] = sl
                wts[o:o + n] = w
                msrc[o:o + n] = s
                o += int(Csec[tt, hh]) * 128
        F1 = cfg.H2 + 1
        m1 = np.ones((TOKT, F1), np.float32)
        m1[:, :cfg.H2] = xp[msrc]
        m1[wts == 0, :cfg.H2] = 0
        m1 = m1.astype(ml_dtypes.bfloat16)
        per_core.append({
            "gidx": wrap16(gidx),
            "dstloc": dstloc.reshape(-1, 128).T.astype(ml_dtypes.bfloat16).copy(),
            "wts": wts.reshape(-1, 128).T.astype(ml_dtypes.bfloat16).copy(),
            # m1 layout [128 part, C_total*F1]: token (ci,p) row at [p, ci*F1:]
            "m1": np.ascontiguousarray(
                m1.reshape(C_total, 128, F1).transpose(1, 0, 2)
                .reshape(128, C_total * F1)),
        })

    plan = {
        "Csec": Csec, "C_total": C_total, "TOKT": TOKT, "calls": calls,
        "chunk_tile": chunk_tile, "chunk_call": chunk_call, "phys": phys,
    }
    return plan, per_core


def build_bass2(cfg, plan):
    P, NT, NTP, NPHYS = cfg.P, cfg.NT, cfg.NTP, cfg.NPHYS
    NTPE = cfg.NTP_EXT
    H2, H1, D, half, eps, N = cfg.H2, cfg.H1, cfg.D, cfg.half, cfg.eps, cfg.N
    Csec, C_total, TOKT = plan["Csec"], plan["C_total"], plan["TOKT"]
    calls = plan["calls"]
    CmaxT = int((Csec[:, 0] + Csec[:, 1]).max())     # L1 stream tile chunks
    CmaxC = max(ntok // 128 for (_, _, ntok, _, _) in calls)  # L2 call chunks

    nc = bacc.Bacc("TRN2", target_bir_lowering=False, debug=False,
                   num_swdge_queues=cfg.nq)

    m1 = nc.dram_tensor("m1", [128, C_total * (H2 + 1)], BF16, kind="ExternalInput")
    gidx = nc.dram_tensor("gidx", [128, TOKT // 16], I16, kind="ExternalInput")
    dstloc = nc.dram_tensor("dstloc", [128, C_total], BF16, kind="ExternalInput")
    wts = nc.dram_tensor("wts", [128, C_total], BF16, kind="ExternalInput")
    iota_in = nc.dram_tensor("iota", [128, 128], BF16, kind="ExternalInput")
    mask16 = nc.dram_tensor("mask16", [16, 2], BF16, kind="ExternalInput")
    w1 = nc.dram_tensor("w1", [H2, H1], F32, kind="ExternalInput")
    w2 = nc.dram_tensor("w2", [H1, D], F32, kind="ExternalInput")
    gam1 = nc.dram_tensor("gam1", [H1], F32, kind="ExternalInput")
    bet1 = nc.dram_tensor("bet1", [H1], F32, kind="ExternalInput")
    gam2 = nc.dram_tensor("gam2", [D], F32, kind="ExternalInput")
    bet2 = nc.dram_tensor("bet2", [D], F32, kind="ExternalInput")
    out = nc.dram_tensor("out", [NTP, D], F32, kind="ExternalOutput")

    h1my = nc.dram_tensor("h1my", [NTPE, H1], BF16)
    h1full = nc.dram_tensor("h1full", [NPHYS, H1], BF16, addr_space="Shared")
    bn2_in = nc.dram_tensor("bn2_in", [1, 2 * D], F32)
    bn2_out = nc.dram_tensor("bn2_out", [1, 2 * D], F32, addr_space="Shared")

    groups = [list(range(P))]

    with tile.TileContext(nc) as tc:
        with (
            tc.tile_pool(name="const", bufs=1) as constp,
            tc.tile_pool(name="stream", bufs=3) as sp,
            tc.tile_pool(name="gath", bufs=3) as gp,
            tc.tile_pool(name="mm", bufs=3) as mp,
            tc.tile_pool(name="dense", bufs=2) as dp,
            tc.tile_pool(name="bn", bufs=1) as bnp,
            tc.tile_pool(name="psz", bufs=3, space="PSUM") as ppz,
            tc.tile_pool(name="psh", bufs=2, space="PSUM") as pph,
            tc.tile_pool(name="psb", bufs=1, space="PSUM") as ppb,
        ):
            # ---- resident constants ----
            iota_sb = constp.tile([128, 128], BF16)
            nc.sync.dma_start(out=iota_sb[:], in_=iota_in.ap())
            dst_sb = constp.tile([128, C_total], BF16)
            nc.sync.dma_start(out=dst_sb[:], in_=dstloc.ap())
            w_sb = constp.tile([128, C_total], BF16)
            nc.sync.dma_start(out=w_sb[:], in_=wts.ap())
            gi_sb = constp.tile([128, TOKT // 16], I16)
            w1_sb = constp.tile([H2, H1], F32)
            nc.sync.dma_start(out=w1_sb[:], in_=w1.ap())
            w2_sb = constp.tile([H1, D], F32)
            ones = constp.tile([128, 1], F32)
            nc.vector.memset(ones[:], 1.0)
            eps_t = constp.tile([128, 1], F32)
            nc.vector.memset(eps_t[:], eps)
            ident = constp.tile([128, 128], F32)
            make_identity(nc, ident[:])
            gb1 = constp.tile([2, H1], F32)
            nc.sync.dma_start(out=gb1[0:1, :], in_=gam1.ap().unsqueeze(0))
            nc.sync.dma_start(out=gb1[1:2, :], in_=bet1.ap().unsqueeze(0))
            mask16_sb = constp.tile([16, 2], BF16)
            nc.sync.dma_start(out=mask16_sb[:], in_=mask16.ap())
            gam2_b = constp.tile([128, D], F32)
            bet2_b = constp.tile([128, D], F32)
            nc.sync.dma_start(out=gam2_b[:], in_=gam2.ap().unsqueeze(0).broadcast_to([128, D]))
            nc.sync.dma_start(out=bet2_b[:], in_=bet2.ap().unsqueeze(0).broadcast_to([128, D]))
            acc1 = constp.tile([128, 2 * H1], F32)
            acc2 = constp.tile([128, 2 * D], F32)
            nc.vector.memset(acc1[:], 0.0)
            nc.vector.memset(acc2[:], 0.0)
            hraw2 = constp.tile([128, NT * D], F32)
            degw_all = constp.tile([1, NT * 128], F32)

            # tile -> (chunk offsets): rebuild per-tile schedule
            tile_chunks = [[] for _ in range(NT)]
            for gci, tt in enumerate(plan["chunk_tile"]):
                tile_chunks[tt].append(gci)
            call_of_chunk = plan["chunk_call"]

            def layer(lnum, Fin, Fout, w_t, acc, hraw, table, s2v_sb=None):
                qsel = [0]
                loaded_call = [-1]
                cur = {}

                def build_M(c0, C, tag):
                    Mb = mp.tile([128, CmaxT if tag == "Mb1" else CmaxC, 128],
                                 BF16, tag=tag)
                    dbc = dst_sb[:, c0:c0 + C].unsqueeze(-1).broadcast_to([128, C, 128])
                    wbc = w_sb[:, c0:c0 + C].unsqueeze(-1).broadcast_to([128, C, 128])
                    ibc = iota_sb[:].unsqueeze(1).broadcast_to([128, C, 128])
                    nc.vector.tensor_tensor(Mb[:, :C, :], dbc, ibc, ALU.is_equal)
                    nc.vector.tensor_tensor(Mb[:, :C, :], Mb[:, :C, :], wbc, ALU.mult)
                    return Mb

                Fst = Fin + 1 if lnum == 1 else Fin

                def prefetch1(tt):
                    chs = tile_chunks[tt]
                    c0 = chs[0]
                    nch = len(chs)
                    st = sp.tile([128, CmaxT, Fst], BF16, tag="m1t")
                    nc.sync.dma_start(
                        out=st[:, :nch, :],
                        in_=m1.ap()[:, c0 * Fst:(c0 + nch) * Fst]
                            .rearrange("p (c f) -> p c f", f=Fst))
                    return st, build_M(c0, nch, "Mb1")

                pend = []
                pf = []
                if lnum == 1:
                    pf.append(prefetch1(0))
                    if NT > 1:
                        pf.append(prefetch1(1))
                for tt in range(NT):
                    chs = tile_chunks[tt]
                    if not chs:
                        continue
                    zt_full = ppz.tile([128, 128], F32, tag="zt")
                    zt_ps = zt_full[:Fst, :]
                    if lnum == 1:
                        st, Mb1 = pf.pop(0)
                        if tt + 2 < NT:
                            pf.append(prefetch1(tt + 2))
                    for ki, gci in enumerate(chs):
                        if lnum == 2:
                            ci = call_of_chunk[gci]
                            if ci != loaded_call[0]:
                                hh, tok_off, ntok, _, _ = calls[ci]
                                gt = gp.tile([128, CmaxC, Fin], BF16, tag="g2")
                                src_ap = table.ap()[half:, :] if hh else table.ap()
                                nc.gpsimd.dma_gather(
                                    out_ap=gt[:, :ntok // 128, :], in_ap=src_ap,
                                    idxs_ap=gi_sb[:, tok_off // 16:(tok_off + ntok) // 16],
                                    num_idxs=ntok, num_idxs_reg=ntok,
                                    elem_size=Fin, queue_num=qsel[0],
                                    single_packet=False)
                                qsel[0] = (qsel[0] + 1) % cfg.nq
                                loaded_call[0] = ci
                                cur["tile"] = gt
                                cur["base"] = gci
                                cur["M"] = build_M(gci, ntok // 128, "Mb2")
                            msg = cur["tile"][:, gci - cur["base"], :]
                            M = cur["M"][:, gci - cur["base"], :]
                        else:
                            msg = st[:, ki, :]
                            M = Mb1[:, ki, :]
                        nc.tensor.matmul(zt_ps, msg, M,
                                         start=(ki == 0), stop=(ki == len(chs) - 1))
                    # dense: H = relu(Z @ W) (+ degw x s2v correction in L2)
                    if lnum == 1:
                        pend.append((tt, zt_full))
                        if len(pend) < 3:
                            continue
                        tt, zt_full = pend.pop(0)
                        zt_ps = zt_full[:Fst, :]
                    ztsb_full = dp.tile([128, 128], F32, tag="ztsb")
                    nc.scalar.copy(ztsb_full[:Fst, :], zt_ps)
                    if lnum == 1:
                        nc.sync.dma_start(
                            out=degw_all[:, tt * 128:(tt + 1) * 128],
                            in_=ztsb_full[Fin:Fin + 1, :])
                    h_full = pph.tile([128, 256], F32, tag="h")
                    h_ps = h_full[:, :Fout]
                    nc.tensor.matmul(h_ps, ztsb_full[:Fin, :], w_t[:],
                                     start=True, stop=(lnum == 1))
                    if lnum == 2:
                        nc.tensor.matmul(h_ps, degw_all[:, tt * 128:(tt + 1) * 128],
                                         s2v_sb[:], start=False, stop=True)
                    hview = hraw[:, tt * Fout:(tt + 1) * Fout] if lnum == 2 else None
                    if lnum == 1:
                        hb = dp.tile([128, H1], BF16, tag="hbf")
                        nc.scalar.activation(hb[:], h_ps, AF.Relu)
                        sq = dp.tile([128, Fout], F32, tag="sq")
                        nc.scalar.activation(sq[:], hb[:], AF.Square)
                        nc.vector.tensor_add(acc[:, :Fout], acc[:, :Fout], hb[:])
                        nc.vector.tensor_add(acc[:, Fout:], acc[:, Fout:], sq[:])
                        nc.sync.dma_start(
                            out=h1my.ap()[tt * 128:(tt + 1) * 128, :], in_=hb[:])
                        if tt == cfg.SPLIT_T - 1:
                            nc.gpsimd.collective_compute(
                                "AllGather", ALU.bypass, replica_groups=groups,
                                ins=[h1my.ap()[:cfg.S1, :]],
                                outs=[h1full.ap()[:P * cfg.S1, :]])
                    else:
                        nc.scalar.activation(hview, h_ps, AF.Relu)
                        sq = dp.tile([128, Fout], F32, tag="sq")
                        nc.scalar.activation(sq[:], hview, AF.Square)
                        nc.vector.tensor_add(acc[:, :Fout], acc[:, :Fout], hview)
                        nc.vector.tensor_add(acc[:, Fout:], acc[:, Fout:], sq[:])
                while lnum == 1 and pend:
                    tt, zt_full = pend.pop(0)
                    zt_ps = zt_full[:Fst, :]
                    ztsb_full = dp.tile([128, 128], F32, tag="ztsb")
                    nc.scalar.copy(ztsb_full[:Fst, :], zt_ps)
                    nc.sync.dma_start(
                        out=degw_all[:, tt * 128:(tt + 1) * 128],
                        in_=ztsb_full[Fin:Fin + 1, :])
                    h_full = pph.tile([128, 256], F32, tag="h")
                    h_ps = h_full[:, :Fout]
                    nc.tensor.matmul(h_ps, ztsb_full[:Fin, :], w_t[:],
                                     start=True, stop=True)
                    hb = dp.tile([128, H1], BF16, tag="hbf")
                    nc.scalar.activation(hb[:], h_ps, AF.Relu)
                    sq = dp.tile([128, Fout], F32, tag="sq")
                    nc.scalar.activation(sq[:], hb[:], AF.Square)
                    nc.vector.tensor_add(acc[:, :Fout], acc[:, :Fout], hb[:])
                    nc.vector.tensor_add(acc[:, Fout:], acc[:, Fout:], sq[:])
                    nc.sync.dma_start(
                        out=h1my.ap()[tt * 128:(tt + 1) * 128, :], in_=hb[:])
                    if tt == cfg.SPLIT_T - 1:
                        nc.gpsimd.collective_compute(
                            "AllGather", ALU.bypass, replica_groups=groups,
                            ins=[h1my.ap()[:cfg.S1, :]],
                            outs=[h1full.ap()[:P * cfg.S1, :]])

            def bn_stats(F, acc, bn_in, bn_out, tag):
                st_ps = ppb.tile([1, 512], F32, tag="st")
                nc.tensor.matmul(st_ps[:, :2 * F], ones[:], acc[:], start=True, stop=True)
                st_sb = dp.tile([1, 2 * F], F32, tag=f"st{tag}")
                nc.scalar.copy(st_sb[:], st_ps[:, :2 * F])
                nc.sync.dma_start(out=bn_in.ap(), in_=st_sb[:])
                nc.gpsimd.collective_compute(
                    "AllReduce", ALU.add, replica_groups=groups,
                    ins=[bn_in.ap()], outs=[bn_out.ap()])
                stb = bnp.tile([128, 2 * F], F32, tag=f"stb{tag}")
                nc.sync.dma_start(out=stb[:], in_=bn_out.ap().broadcast_to([128, 2 * F]))
                mean = bnp.tile([128, F], F32, tag=f"mean{tag}")
                var = bnp.tile([128, F], F32, tag=f"var{tag}")
                nc.scalar.mul(mean[:], stb[:, :F], 1.0 / N)
                nc.scalar.mul(var[:], stb[:, F:], 1.0 / N)
                msq = bnp.tile([128, F], F32, tag=f"msq{tag}")
                nc.scalar.activation(msq[:], mean[:], AF.Square)
                nc.vector.tensor_sub(var[:], var[:], msq[:])
                sd = bnp.tile([128, F], F32, tag=f"sd{tag}")
                nc.scalar.activation(sd[:], var[:], AF.Sqrt, bias=eps_t[:])
                inv = bnp.tile([128, F], F32, tag=f"inv{tag}")
                nc.vector.reciprocal(inv[:], sd[:])
                return mean, inv

            # ================= layer 1 =================
            layer(1, H2, H1, w1_sb, acc1, None, None)
            nc.sync.dma_start(out=gi_sb[:], in_=gidx.ap())
            nc.sync.dma_start(out=w2_sb[:], in_=w2.ap())
            # append this core's BN1 partial sums as 2 bf16 rows of h1my
            st_ps = ppb.tile([1, 512], F32, tag="st")
            nc.tensor.matmul(st_ps[:, :2 * H1], ones[:], acc1[:], start=True, stop=True)
            st_bf = dp.tile([1, 2 * H1], BF16, tag="stbf")
            nc.scalar.copy(st_bf[:], st_ps[:, :2 * H1])
            nc.sync.dma_start(out=h1my.ap()[NTP:NTP + 1, :], in_=st_bf[:, :H1])
            nc.sync.dma_start(out=h1my.ap()[NTP + 1:NTP + 2, :], in_=st_bf[:, H1:])
            nc.gpsimd.collective_compute(
                "AllGather", ALU.bypass, replica_groups=groups,
                ins=[h1my.ap()[cfg.S1:, :]],
                outs=[h1full.ap()[P * cfg.S1:, :]])
            # combine the 8 partial-stat row pairs locally (no AllReduce)
            stat_off = P * cfg.S1 + (NTP - cfg.S1)
            statall = bnp.tile([16, H1], BF16, tag="statall")
            for cc in range(P):
                nc.sync.dma_start(
                    out=statall[2 * cc:2 * cc + 2, :],
                    in_=h1full.ap()[stat_off + cc * cfg.S2:
                                    stat_off + cc * cfg.S2 + 2, :])
            stred_ps = ppb.tile([2, H1], F32, tag="str")
            nc.tensor.matmul(stred_ps[:], mask16_sb[:], statall[:], start=True, stop=True)
            stsb = dp.tile([2, H1], F32, tag="stsb")
            nc.scalar.copy(stsb[:], stred_ps[:])
            tr_ps = ppb.tile([128, 4], F32, tag="tr")
            nc.tensor.transpose(tr_ps[:H1, 0:2], stsb[:], ident[:2, 0:2])
            colstats = bnp.tile([H1, 2], F32, tag="colstats")
            nc.scalar.copy(colstats[:], tr_ps[:H1, 0:2])
            nc.tensor.transpose(tr_ps[:H1, 2:4], gb1[:], ident[:2, 0:2])
            gbc = bnp.tile([H1, 2], F32, tag="gbc")
            nc.scalar.copy(gbc[:], tr_ps[:H1, 2:4])
            gam1_c = gbc[:, 0:1]
            bet1_c = gbc[:, 1:2]
            mean_c = bnp.tile([H1, 1], F32, tag="mean_c")
            var_c = bnp.tile([H1, 1], F32, tag="var_c")
            nc.scalar.mul(mean_c[:], colstats[:, 0:1], 1.0 / N)
            nc.scalar.mul(var_c[:], colstats[:, 1:2], 1.0 / N)
            msq_c = bnp.tile([H1, 1], F32, tag="msq_c")
            nc.scalar.activation(msq_c[:], mean_c[:], AF.Square)
            nc.vector.tensor_sub(var_c[:], var_c[:], msq_c[:])
            sd_c = bnp.tile([H1, 1], F32, tag="sd_c")
            nc.scalar.activation(sd_c[:], var_c[:], AF.Sqrt, bias=eps_t[:H1])
            inv_c = bnp.tile([H1, 1], F32, tag="inv_c")
            nc.vector.reciprocal(inv_c[:], sd_c[:])
            scale_c = bnp.tile([H1, 1], F32, tag="scale_c")
            shift_c = bnp.tile([H1, 1], F32, tag="shift_c")
            nc.vector.tensor_mul(scale_c[:], gam1_c, inv_c[:])
            nc.vector.tensor_mul(shift_c[:], mean_c[:], scale_c[:])
            nc.vector.tensor_sub(shift_c[:], bet1_c, shift_c[:])
            w2p = constp.tile([H1, D], F32)
            nc.vector.tensor_scalar_mul(w2p[:], w2_sb[:], scale_c[:])
            s2v_ps = ppb.tile([1, 512], F32, tag="st")
            nc.tensor.matmul(s2v_ps[:, :D], shift_c[:], w2_sb[:], start=True, stop=True)
            s2v_sb = bnp.tile([1, D], F32, tag="s2v")
            nc.scalar.copy(s2v_sb[:], s2v_ps[:, :D])
            # ================= layer 2 =================
            layer(2, H1, D, w2p, acc2, hraw2, h1full, s2v_sb=s2v_sb)
            mean2, inv2 = bn_stats(D, acc2, bn2_in, bn2_out, "2")
            scale2 = bnp.tile([128, D], F32, tag="scale2")
            shift2 = bnp.tile([128, D], F32, tag="shift2")
            nc.vector.tensor_mul(scale2[:], gam2_b[:], inv2[:])
            nc.vector.tensor_mul(shift2[:], mean2[:], scale2[:])
            nc.vector.tensor_sub(shift2[:], bet2_b[:], shift2[:])
            GAPP = 13
            for g0 in range(0, NT, GAPP):
                gn = min(GAPP, NT - g0)
                hv = hraw2[:, g0 * D:(g0 + gn) * D].rearrange(
                    "p (t f) -> p t f", f=D)
                t2 = dp.tile([128, GAPP, D], F32, tag="bna2")
                sc_b = scale2[:].unsqueeze(1).broadcast_to([128, gn, D])
                sh_b = shift2[:].unsqueeze(1).broadcast_to([128, gn, D])
                nc.vector.tensor_mul(t2[:, :gn, :], hv, sc_b)
                nc.vector.tensor_add(t2[:, :gn, :], t2[:, :gn, :], sh_b)
                nc.sync.dma_start(
                    out=out.ap()[g0 * 128:(g0 + gn) * 128, :]
                        .rearrange("(t p) f -> p t f", p=128),
                    in_=t2[:, :gn, :])

    nc.compile()
    return nc


def make_in_maps2(cfg, plan, per_core, inputs):
    iota = np.tile(np.arange(128, dtype=np.float32), (128, 1)).astype(ml_dtypes.bfloat16)
    mask16 = np.zeros((16, 2), np.float32)
    mask16[np.arange(16), np.arange(16) % 2] = 1.0
    shared = {
        "iota": iota,
        "mask16": mask16.astype(ml_dtypes.bfloat16),
        "w1": np.ascontiguousarray(inputs["W1"], dtype=np.float32),
        "w2": np.ascontiguousarray(inputs["W2"], dtype=np.float32),
        "gam1": np.ascontiguousarray(inputs["gamma1"], dtype=np.float32),
        "bet1": np.ascontiguousarray(inputs["beta1"], dtype=np.float32),
        "gam2": np.ascontiguousarray(inputs["gamma2"], dtype=np.float32),
        "bet2": np.ascontiguousarray(inputs["beta2"], dtype=np.float32),
    }
    return [{**shared, **per_core[c]} for c in range(cfg.P)]


def run2(cfg, inputs, use_hw=True, trace=False):
    plan, per_core = build_plan2(
        cfg, inputs["edge_src"], inputs["edge_dst"], inputs["edge_weight"],
        inputs["y_features"])
    print(f"plan2: C_total={plan['C_total']} chunks ({plan['TOKT']} tokens, "
          f"real {cfg.E // cfg.P} avg), {len(plan['calls'])} calls/layer")
    nc = build_bass2(cfg, plan)
    in_maps = make_in_maps2(cfg, plan, per_core, inputs)
    phys = plan["phys"]
    if use_hw:
        from concourse.bass_utils import run_bass_kernel_spmd
        res = run_bass_kernel_spmd(nc, in_maps, list(range(cfg.P)), trace=trace)
        outs = np.concatenate([res.results[c]["out"] for c in range(cfg.P)], 0)
    else:
        from concourse.bass_interp import MultiCoreSim
        sim = MultiCoreSim(nc, num_cores=cfg.P, require_finite=False,
                           require_nnan=False)
        for c in range(cfg.P):
            for k, v in in_maps[c].items():
                sim.cores[c].tensor(k)[:] = v
            sim.cores[c].tensor("out")[:] = 0
        sim.simulate()
        outs = np.concatenate(
            [np.array(sim.cores[c].mem_tensor("out")) for c in range(cfg.P)], 0)
        res = None
    in_s1 = phys < cfg.P * cfg.S1
    core = np.where(in_s1, phys // cfg.S1, (phys - cfg.P * cfg.S1) // cfg.S2)
    row = np.where(in_s1, phys % cfg.S1,
                   cfg.S1 + (phys - cfg.P * cfg.S1) % cfg.S2)
    idx = core * cfg.NTP + row
    return np.ascontiguousarray(outs[idx], dtype=np.float32), res


# ======================= harness entry point =======================

_CFG = Cfg2(N=50000, E=800000, H2=64, H1=128, D=256, P=8,
            half=32768, cap=1024, nq=2)


def kernel(**inputs) -> np.ndarray:
    """Full-input, full-output 2-layer GCN forward on 8 TRN2 NeuronCores."""
    out, _ = run2(_CFG, inputs, use_hw=True, trace=False)
    return np.ascontiguousarray(out, dtype=np.float32)
